# revision 1
# baseline (speedup 1.0000x reference)
"""TRN2 Bass kernel for nn_BasicBlock_85761906966887 (sparse conv basic block).

Structure (per NeuronCore, 8 cores, point-sharded):
  conv1: rulebook gather (indirect DMA, 128 rows/call) -> PE transpose ->
         bf16 matmul accumulate over 27 offsets -> instance-norm stats
  AllReduce stats; normalize + leaky; transpose rows; AllGather h1 table
  conv2: same against h1 table; norm2 + residual + leaky -> output shard
Host: shards/clamps rulebook indices, casts tables to bf16, reassembles output.
"""
import sys

for p in ("/opt/trn_rl_repo", "/root/.axon_site/_ro/trn_rl_repo"):
    if p not in sys.path:
        sys.path.insert(0, p)

import numpy as np
import ml_dtypes

import concourse.bass as bass
import concourse.bacc as bacc
import concourse.mybir as mybir
import concourse.tile as tile
from concourse.masks import make_identity

N = 131072
C = 96
K = 27
NCORES = 8
SHARD = N // NCORES          # 16384 points per core
GROUPS = SHARD // 512        # 32 groups of 512 points
JT = 4                       # 128-point tiles per group
SLOT = K * JT                # 108 gather calls per group
ZROW = N                     # zero row index in padded tables
EPS = 1e-5
SLOPE = 0.01

F32 = mybir.dt.float32
BF16 = mybir.dt.bfloat16
I32 = mybir.dt.int32
BF = ml_dtypes.bfloat16


NQ = 4        # SWDGE queues to spread indirect gathers across
CENTER = K // 2


def build_program():
    nc = bacc.Bacc("TRN2", target_bir_lowering=False, debug=False,
                   num_devices=NCORES, num_swdge_queues=NQ)

    # ---------------- I/O ----------------
    xpad = nc.dram_tensor("xpad", [N + 1, C], BF16, kind="ExternalInput")
    xself = nc.dram_tensor("xself", [SHARD, C], BF16, kind="ExternalInput")
    idx1 = nc.dram_tensor("idx1", [GROUPS * 128, SLOT], I32, kind="ExternalInput")
    idx2 = nc.dram_tensor("idx2", [GROUPS * 128, SLOT], I32, kind="ExternalInput")
    w1 = nc.dram_tensor("w1", [C, K * C], BF16, kind="ExternalInput")
    w2 = nc.dram_tensor("w2", [C, K * C], BF16, kind="ExternalInput")
    xt = nc.dram_tensor("xt", [C, SHARD], F32, kind="ExternalInput")
    gb1 = nc.dram_tensor("gb1", [C, 2], F32, kind="ExternalInput")  # gamma1, beta1
    gb2 = nc.dram_tensor("gb2", [C, 2], F32, kind="ExternalInput")
    out = nc.dram_tensor("out", [C, SHARD], F32, kind="ExternalOutput")

    with tile.TileContext(nc) as tc:
        with tc.tile_pool(name="persist", bufs=1) as pp, \
             tc.tile_pool(name="gpool", bufs=32) as gp, \
             tc.tile_pool(name="gtpool", bufs=3) as gtp, \
             tc.tile_pool(name="iopool", bufs=2) as iop, \
             tc.tile_pool(name="mmpsum", bufs=2, space="PSUM") as mmp, \
             tc.tile_pool(name="tppsum", bufs=3, space="PSUM") as tpp, \
             tc.tile_pool(name="wpsum", bufs=2, space="PSUM") as wpp, \
             tc.tile_pool(name="dram", bufs=1, space="DRAM") as dp:

            ident = pp.tile([128, 128], BF16)
            make_identity(nc, ident[:])
            w1t = pp.tile([C, K * C], BF16)
            nc.sync.dma_start(out=w1t[:], in_=w1[:])
            w2t = pp.tile([C, K * C], BF16)
            nc.sync.dma_start(out=w2t[:], in_=w2[:])
            gb1t = pp.tile([C, 2], F32)
            nc.sync.dma_start(out=gb1t[:], in_=gb1[:])
            gb2t = pp.tile([C, 2], F32)
            nc.sync.dma_start(out=gb2t[:], in_=gb2[:])

            # raw conv output store, shared between the two convs
            raw = pp.tile([C, SHARD], BF16)
            sums = pp.tile([C, GROUPS], F32)
            ssums = pp.tile([C, GROUPS], F32)

            # DRAM internals
            h1shard = dp.tile([SHARD, C], BF16)
            h1full = dp.tile([N, C], BF16, addr_space="Shared")
            h1tab = dp.tile([N + 1, C], BF16)
            st_in1 = dp.tile([C, 2], F32)
            st_out1 = dp.tile([C, 2], F32, addr_space="Shared")
            st_in2 = dp.tile([C, 2], F32)
            st_out2 = dp.tile([C, 2], F32, addr_space="Shared")

            def conv_pass(table_ap, self_rows, idx_dram, wt):
                """Gather+matmul conv; fills raw/sums/ssums.
                self_rows: [SHARD, C] rows of this core's own points (center k)."""
                with tc.For_i(0, GROUPS, 1) as iv:
                    it = iop.tile([128, SLOT], I32, tag="idx", name="it")
                    nc.sync.dma_start(
                        out=it[:], in_=idx_dram[bass.ds(iv * 128, 128)])
                    mm = mmp.tile([C, 512], F32, name="mm")
                    for k in range(K):
                        tp = tpp.tile([C, 512], BF16, tag="tp", name=f"tp{k}")
                        for j in range(JT):
                            s = k * JT + j
                            g = gp.tile([128, C], BF16, tag="g", name=f"g{s}")
                            if k == CENTER:
                                # center offset is the point itself: dense rows
                                nc.sync.dma_start(
                                    out=g[:],
                                    in_=self_rows[bass.ds(iv * 512 + j * 128, 128)])
                            else:
                                gi = nc.gpsimd.indirect_dma_start(
                                    out=g[:], out_offset=None, in_=table_ap,
                                    in_offset=bass.IndirectOffsetOnAxis(
                                        ap=it[:, s:s + 1], axis=0))
                                gi.ins.queue = f"qPoolDynamic{(s % NQ) or ''}" \
                                    if (s % NQ) else "qPoolDynamic"
                            nc.tensor.matmul(
                                out=tp[:, j * 128:(j + 1) * 128],
                                lhsT=g[:], rhs=ident[:],
                                is_transpose=True,
                                start=(j == 0), stop=(j == JT - 1))
                        gt = gtp.tile([C, 512], BF16, tag="gt", name=f"gt{k}")
                        if k % 2 == 0:
                            nc.vector.tensor_copy(out=gt[:], in_=tp[:])
                        else:
                            nc.scalar.activation(
                                out=gt[:], in_=tp[:],
                                func=mybir.ActivationFunctionType.Identity)
                        nc.tensor.matmul(
                            out=mm[:], lhsT=wt[:, k * C:(k + 1) * C], rhs=gt[:],
                            start=(k == 0), stop=(k == K - 1))
                    # stats + raw store (ACT, reads PSUM)
                    nc.scalar.activation(
                        out=raw[:, bass.ds(iv * 512, 512)], in_=mm[:],
                        func=mybir.ActivationFunctionType.Identity,
                        accum_out=sums[:, bass.ds(iv, 1)])
                    junk = iop.tile([C, 512], BF16, tag="junk", name="junk")
                    nc.scalar.activation(
                        out=junk[:], in_=mm[:],
                        func=mybir.ActivationFunctionType.Square,
                        accum_out=ssums[:, bass.ds(iv, 1)])

            def stats_allreduce(gbt, st_in, st_out):
                """Returns (scale, nbias) [C,1] tiles from sums/ssums + AllReduce."""
                loc = pp.tile([C, 2], F32, name="loc", uniquify=True)
                nc.vector.reduce_sum(loc[:, 0:1], sums[:], axis=mybir.AxisListType.X)
                nc.vector.reduce_sum(loc[:, 1:2], ssums[:], axis=mybir.AxisListType.X)
                nc.gpsimd.dma_start(out=st_in[:], in_=loc[:])
                nc.gpsimd.collective_compute(
                    "AllReduce", mybir.AluOpType.add,
                    replica_groups=[list(range(NCORES))],
                    ins=[st_in[:]], outs=[st_out[:]])
                glob = pp.tile([C, 2], F32, name="glob", uniquify=True)
                nc.gpsimd.dma_start(out=glob[:], in_=st_out[:])
                # mu = sum/N ; var = ss/N - mu^2 ; scale = gamma/sqrt(var+eps)
                # nbias = beta - mu*scale
                sc = pp.tile([C, 6], F32, name="sc", uniquify=True)
                nc.scalar.activation(out=sc[:, 0:1], in_=glob[:, 0:1],
                                     func=mybir.ActivationFunctionType.Identity,
                                     scale=1.0 / N)  # mu
                nc.scalar.activation(out=sc[:, 1:2], in_=glob[:, 1:2],
                                     func=mybir.ActivationFunctionType.Identity,
                                     scale=1.0 / N)  # E[x^2]
                nc.scalar.activation(out=sc[:, 2:3], in_=sc[:, 0:1],
                                     func=mybir.ActivationFunctionType.Square)
                nc.vector.tensor_tensor(out=sc[:, 3:4], in0=sc[:, 1:2],
                                        in1=sc[:, 2:3],
                                        op=mybir.AluOpType.subtract)  # var
                nc.vector.tensor_scalar_add(sc[:, 3:4], sc[:, 3:4], EPS)
                nc.scalar.activation(out=sc[:, 4:5], in_=sc[:, 3:4],
                                     func=mybir.ActivationFunctionType.Sqrt)
                nc.vector.reciprocal(sc[:, 5:6], sc[:, 4:5])  # rstd
                res = pp.tile([C, 2], F32, name="res", uniquify=True)
                nc.vector.tensor_tensor(out=res[:, 0:1], in0=gbt[:, 0:1],
                                        in1=sc[:, 5:6],
                                        op=mybir.AluOpType.mult)  # scale
                tmp = pp.tile([C, 1], F32, name="tmp", uniquify=True)
                nc.vector.tensor_tensor(out=tmp[:], in0=sc[:, 0:1],
                                        in1=res[:, 0:1], op=mybir.AluOpType.mult)
                nc.vector.tensor_tensor(out=res[:, 1:2], in0=gbt[:, 1:2],
                                        in1=tmp[:], op=mybir.AluOpType.subtract)
                return res

            # ---------------- conv1 ----------------
            conv_pass(xpad[:], xself[:], idx1, w1t)
            res1 = stats_allreduce(gb1t, st_in1, st_out1)

            # normalize + leaky + transpose to rows -> h1shard
            for t in range(GROUPS):
                z = iop.tile([C, 512], F32, tag="z", name=f"z{t}")
                nc.scalar.activation(out=z[:], in_=raw[:, t * 512:(t + 1) * 512],
                                     func=mybir.ActivationFunctionType.Identity,
                                     scale=res1[:, 0:1], bias=res1[:, 1:2])
                zs = iop.tile([C, 512], F32, tag="zs", name=f"zs{t}")
                nc.vector.tensor_scalar_mul(zs[:], z[:], SLOPE)
                h1n = iop.tile([C, 512], BF16, tag="h1n", name=f"h1n{t}")
                nc.vector.tensor_tensor(out=h1n[:], in0=z[:], in1=zs[:],
                                        op=mybir.AluOpType.max)
                h1r = iop.tile([128, JT * C], BF16, tag="h1r", name=f"h1r{t}")
                for j in range(JT):
                    wp = wpp.tile([128, C], BF16, tag="wp", name=f"wp{t}_{j}")
                    nc.tensor.matmul(out=wp[:], lhsT=h1n[:, j * 128:(j + 1) * 128],
                                     rhs=ident[:C, :C], is_transpose=True,
                                     start=True, stop=True)
                    if j % 2 == 0:
                        nc.vector.tensor_copy(out=h1r[:, j * C:(j + 1) * C], in_=wp[:])
                    else:
                        nc.scalar.activation(
                            out=h1r[:, j * C:(j + 1) * C], in_=wp[:],
                            func=mybir.ActivationFunctionType.Identity)
                nc.sync.dma_start(
                    out=h1shard[t * 512:(t + 1) * 512, :].rearrange(
                        "(j p) c -> p j c", p=128),
                    in_=h1r[:].rearrange("p (j c) -> p j c", c=C))

            # AllGather h1 then copy into padded gather table with zero row
            nc.gpsimd.collective_compute(
                "AllGather", mybir.AluOpType.bypass,
                replica_groups=[list(range(NCORES))],
                ins=[h1shard[:]], outs=[h1full[:]])
            zrow = pp.tile([1, C], BF16)
            nc.vector.memset(zrow[:], 0.0)
            nc.sync.dma_start(out=h1tab[N:N + 1, :], in_=zrow[:])
            nc.sync.dma_start(out=h1tab[0:N, :], in_=h1full[:])

            # ---------------- conv2 ----------------
            conv_pass(h1tab[:], h1shard[:], idx2, w2t)
            res2 = stats_allreduce(gb2t, st_in2, st_out2)

            # norm2 + residual + leaky -> out
            for t in range(GROUPS):
                z = iop.tile([C, 512], F32, tag="z2", name=f"z2{t}")
                nc.scalar.activation(out=z[:], in_=raw[:, t * 512:(t + 1) * 512],
                                     func=mybir.ActivationFunctionType.Identity,
                                     scale=res2[:, 0:1], bias=res2[:, 1:2])
                xr = iop.tile([C, 512], F32, tag="xr", name=f"xr{t}")
                nc.sync.dma_start(out=xr[:], in_=xt[:, t * 512:(t + 1) * 512])
                r = iop.tile([C, 512], F32, tag="r", name=f"r{t}")
                nc.vector.tensor_add(r[:], z[:], xr[:])
                rs = iop.tile([C, 512], F32, tag="rs", name=f"rs{t}")
                nc.vector.tensor_scalar_mul(rs[:], r[:], SLOPE)
                o = iop.tile([C, 512], F32, tag="o", name=f"o{t}")
                nc.vector.tensor_tensor(out=o[:], in0=r[:], in1=rs[:],
                                        op=mybir.AluOpType.max)
                nc.sync.dma_start(out=out[:, t * 512:(t + 1) * 512], in_=o[:])

    nc.compile()
    return nc


def prep_inputs(x, W1, W2, gamma1, beta1, gamma2, beta2, nbr1, nbr2):
    """Host-side sharding/layout. Returns list of per-core input dicts."""
    xpad = np.zeros((N + 1, C), dtype=BF)
    xpad[:N] = x.astype(BF)
    w1 = np.ascontiguousarray(
        W1.astype(BF).transpose(1, 0, 2).reshape(C, K * C))
    w2 = np.ascontiguousarray(
        W2.astype(BF).transpose(1, 0, 2).reshape(C, K * C))
    gb1 = np.stack([gamma1, beta1], axis=1).astype(np.float32)
    gb2 = np.stack([gamma2, beta2], axis=1).astype(np.float32)

    def prep_idx(nbr):
        cl = np.minimum(np.asarray(nbr), ZROW).astype(np.int32)   # [K, N]

        # -> [cores, groups*128, K*JT] with slot s = k*JT + j mapping
        #    point = c*SHARD + t*512 + j*128 + p
        a = cl.reshape(K, NCORES, GROUPS, JT, 128)
        return np.ascontiguousarray(a.transpose(1, 2, 4, 0, 3)).reshape(
            NCORES, GROUPS * 128, SLOT)

    i1 = prep_idx(nbr1)
    i2 = prep_idx(nbr2)
    xtc = np.ascontiguousarray(
        x.reshape(NCORES, SHARD, C).transpose(0, 2, 1)).astype(np.float32)

    xbf = xpad[:N].reshape(NCORES, SHARD, C)
    maps = []
    for c in range(NCORES):
        maps.append({
            "xpad": xpad, "xself": np.ascontiguousarray(xbf[c]),
            "w1": w1, "w2": w2,
            "gb1": gb1, "gb2": gb2,
            "idx1": i1[c], "idx2": i2[c], "xt": xtc[c],
        })
    return maps


_CACHE = {}


def _get_runner():
    if "r" not in _CACHE:
        from runner_embedded import SpmdRunner
        nc = build_program()
        _CACHE["r"] = SpmdRunner(nc, NCORES)
    return _CACHE["r"]


def kernel(x, W1, W2, gamma1, beta1, gamma2, beta2, nbr1, nbr2):
    x = np.asarray(x, dtype=np.float32)
    maps = prep_inputs(np.asarray(x, np.float32), np.asarray(W1, np.float32),
                       np.asarray(W2, np.float32),
                       np.asarray(gamma1, np.float32), np.asarray(beta1, np.float32),
                       np.asarray(gamma2, np.float32), np.asarray(beta2, np.float32),
                       np.asarray(nbr1), np.asarray(nbr2))
    r = _get_runner()
    res = r.run(r.stage(maps))
    outs = [res[c]["out"] for c in range(NCORES)]          # [C, SHARD] each
    full = np.concatenate([o.T for o in outs], axis=0)     # [N, C]
    return np.ascontiguousarray(full.astype(np.float32))


# ---- embedded minimal SPMD runner (kernel.py must be self-contained) ----
import types

runner_embedded = types.ModuleType("runner_embedded")
exec(
    '''
import time
import numpy as np
import jax
from jax.sharding import Mesh, PartitionSpec
from jax.experimental.shard_map import shard_map
import concourse.bass as bass
import concourse.mybir as mybir
from concourse import bass2jax
from concourse.bass2jax import _bass_exec_p


class SpmdRunner:
    def __init__(self, nc, n_cores):
        bass2jax.install_neuronx_cc_hook()
        self.nc = nc
        self.n_cores = n_cores
        partition_name = nc.partition_id_tensor.name if nc.partition_id_tensor else None
        in_names, out_names, out_avals = [], [], []
        for alloc in nc.m.functions[0].allocations:
            if not isinstance(alloc, mybir.MemoryLocationSet):
                continue
            name = alloc.memorylocations[0].name if alloc.memorylocations else None
            if alloc.kind == "ExternalInput":
                if name != partition_name:
                    in_names.append(name)
            elif alloc.kind == "ExternalOutput":
                out_names.append(name)
                out_avals.append(jax.core.ShapedArray(
                    tuple(alloc.tensor_shape), mybir.dt.np(alloc.dtype)))
        self.in_names, self.out_names, self.out_avals = in_names, out_names, out_avals

        def _body(*args):
            operands = list(args)
            if partition_name is not None:
                operands.append(bass2jax.partition_id_tensor())
            bind_names = list(in_names) + ([partition_name] if partition_name else [])
            outs = _bass_exec_p.bind(
                *operands, out_avals=tuple(out_avals), in_names=tuple(bind_names),
                out_names=tuple(out_names), lowering_input_output_aliases=(),
                sim_require_finite=True, sim_require_nnan=True, nc=nc)
            return tuple(outs)

        devices = jax.devices()[:n_cores]
        assert len(devices) == n_cores
        self.mesh = Mesh(np.asarray(devices), ("core",))
        in_specs = (PartitionSpec("core"),) * len(in_names)
        out_specs = (PartitionSpec("core"),) * len(out_names)
        self.fn = jax.jit(
            shard_map(_body, mesh=self.mesh, in_specs=in_specs,
                      out_specs=out_specs, check_rep=False),
            keep_unused=True)

    def stage(self, in_maps):
        cat = []
        for name in self.in_names:
            arrs = [np.asarray(m[name]) for m in in_maps]
            cat.append(np.concatenate(arrs, axis=0))
        return [jax.device_put(a) for a in cat]

    def run(self, staged):
        outs = self.fn(*staged)
        jax.block_until_ready(outs)
        return [
            {name: np.asarray(outs[i]).reshape(
                self.n_cores, *self.out_avals[i].shape)[c]
             for i, name in enumerate(self.out_names)}
            for c in range(self.n_cores)]

    def time(self, staged, iters=10, warmup=2):
        for _ in range(warmup):
            jax.block_until_ready(self.fn(*staged))
        t0 = time.perf_counter()
        outs = None
        for _ in range(iters):
            outs = self.fn(*staged)
        jax.block_until_ready(outs)
        t1 = time.perf_counter()
        return (t1 - t0) / iters
''',
    runner_embedded.__dict__,
)
sys.modules["runner_embedded"] = runner_embedded



# revision 5
# speedup vs baseline: 4.1720x; 4.1720x over previous
"""TRN2 Bass kernel for nn_BasicBlock_85761906966887 (sparse conv basic block).

Structure (per NeuronCore, 8 cores, point-sharded):
  conv1: rulebook gather (indirect DMA, 128 rows/call) -> PE transpose ->
         bf16 matmul accumulate over 27 offsets -> instance-norm stats
  AllReduce stats; normalize + leaky; transpose rows; AllGather h1 table
  conv2: same against h1 table; norm2 + residual + leaky -> output shard
Host: shards/clamps rulebook indices, casts tables to bf16, reassembles output.
"""
import sys

for p in ("/opt/trn_rl_repo", "/root/.axon_site/_ro/trn_rl_repo"):
    if p not in sys.path:
        sys.path.insert(0, p)

import numpy as np
import ml_dtypes

import concourse.bass as bass
import concourse.bacc as bacc
import concourse.mybir as mybir
import concourse.tile as tile
from concourse.masks import make_identity

N = 131072
C = 96
K = 27
NCORES = 8
SHARD = N // NCORES          # 16384 points per core
GROUPS = SHARD // 512        # 32 groups of 512 points
JT = 4                       # 128-point tiles per group
SLOT = K * JT                # 108 gather calls per group
ZROW = N                     # zero row index in padded tables
EPS = 1e-5
SLOPE = 0.01

F32 = mybir.dt.float32
BF16 = mybir.dt.bfloat16
I32 = mybir.dt.int32
BF = ml_dtypes.bfloat16


NQ = 4        # SWDGE queues to spread indirect gathers across
CENTER = K // 2


def build_program():
    nc = bacc.Bacc("TRN2", target_bir_lowering=False, debug=False,
                   num_devices=NCORES, num_swdge_queues=NQ)

    # ---------------- I/O ----------------
    xpad = nc.dram_tensor("xpad", [N + 1, C], BF16, kind="ExternalInput")
    xself = nc.dram_tensor("xself", [SHARD, C], BF16, kind="ExternalInput")
    idx1 = nc.dram_tensor("idx1", [GROUPS * 128, SLOT], I32, kind="ExternalInput")
    idx2 = nc.dram_tensor("idx2", [GROUPS * 128, SLOT], I32, kind="ExternalInput")
    w1 = nc.dram_tensor("w1", [C, K * C], BF16, kind="ExternalInput")
    w2 = nc.dram_tensor("w2", [C, K * C], BF16, kind="ExternalInput")
    xt = nc.dram_tensor("xt", [C, SHARD], F32, kind="ExternalInput")
    gb1 = nc.dram_tensor("gb1", [C, 2], F32, kind="ExternalInput")  # gamma1, beta1
    gb2 = nc.dram_tensor("gb2", [C, 2], F32, kind="ExternalInput")
    out = nc.dram_tensor("out", [C, SHARD], F32, kind="ExternalOutput")

    with tile.TileContext(nc) as tc:
        with tc.tile_pool(name="persist", bufs=1) as pp, \
             tc.tile_pool(name="gpool", bufs=32) as gp, \
             tc.tile_pool(name="gtpool", bufs=3) as gtp, \
             tc.tile_pool(name="iopool", bufs=2) as iop, \
             tc.tile_pool(name="mmpsum", bufs=2, space="PSUM") as mmp, \
             tc.tile_pool(name="tppsum", bufs=3, space="PSUM") as tpp, \
             tc.tile_pool(name="wpsum", bufs=2, space="PSUM") as wpp, \
             tc.tile_pool(name="dram", bufs=1, space="DRAM") as dp:

            ident = pp.tile([128, 128], BF16)
            make_identity(nc, ident[:])
            w1t = pp.tile([C, K * C], BF16)
            nc.sync.dma_start(out=w1t[:], in_=w1[:])
            w2t = pp.tile([C, K * C], BF16)
            nc.sync.dma_start(out=w2t[:], in_=w2[:])
            gb1t = pp.tile([C, 2], F32)
            nc.sync.dma_start(out=gb1t[:], in_=gb1[:])
            gb2t = pp.tile([C, 2], F32)
            nc.sync.dma_start(out=gb2t[:], in_=gb2[:])

            # raw conv output store, shared between the two convs
            raw = pp.tile([C, SHARD], BF16)
            sums = pp.tile([C, GROUPS], F32)
            ssums = pp.tile([C, GROUPS], F32)

            # DRAM internals
            h1shard = dp.tile([SHARD, C], BF16)
            h1full = dp.tile([N, C], BF16, addr_space="Shared")
            h1tab = dp.tile([N + 1, C], BF16)
            st_in1 = dp.tile([C, 2], F32)
            st_out1 = dp.tile([C, 2], F32, addr_space="Shared")
            st_in2 = dp.tile([C, 2], F32)
            st_out2 = dp.tile([C, 2], F32, addr_space="Shared")

            def conv_pass(table_ap, self_rows, idx_dram, wt):
                """Gather+matmul conv; fills raw/sums/ssums.
                self_rows: [SHARD, C] rows of this core's own points (center k)."""
                with tc.For_i(0, GROUPS, 1) as iv:
                    it = iop.tile([128, SLOT], I32, tag="idx", name="it")
                    nc.sync.dma_start(
                        out=it[:], in_=idx_dram[bass.ds(iv * 128, 128)])
                    mm = mmp.tile([C, 512], F32, name="mm")
                    for k in range(K):
                        tp = tpp.tile([C, 512], BF16, tag="tp", name=f"tp{k}")
                        for j in range(JT):
                            s = k * JT + j
                            g = gp.tile([128, C], BF16, tag="g", name=f"g{s}")
                            if k == CENTER:
                                # center offset is the point itself: dense rows
                                nc.sync.dma_start(
                                    out=g[:],
                                    in_=self_rows[bass.ds(iv * 512 + j * 128, 128)])
                            else:
                                gi = nc.gpsimd.indirect_dma_start(
                                    out=g[:], out_offset=None, in_=table_ap,
                                    in_offset=bass.IndirectOffsetOnAxis(
                                        ap=it[:, s:s + 1], axis=0))
                                gi.ins.queue = f"qPoolDynamic{(s % NQ) or ''}" \
                                    if (s % NQ) else "qPoolDynamic"
                            nc.tensor.matmul(
                                out=tp[:, j * 128:(j + 1) * 128],
                                lhsT=g[:], rhs=ident[:],
                                is_transpose=True,
                                start=(j == 0), stop=(j == JT - 1))
                        gt = gtp.tile([C, 512], BF16, tag="gt", name=f"gt{k}")
                        if k % 2 == 0:
                            nc.vector.tensor_copy(out=gt[:], in_=tp[:])
                        else:
                            nc.scalar.activation(
                                out=gt[:], in_=tp[:],
                                func=mybir.ActivationFunctionType.Identity)
                        nc.tensor.matmul(
                            out=mm[:], lhsT=wt[:, k * C:(k + 1) * C], rhs=gt[:],
                            start=(k == 0), stop=(k == K - 1))
                    # stats + raw store (ACT, reads PSUM)
                    nc.scalar.activation(
                        out=raw[:, bass.ds(iv * 512, 512)], in_=mm[:],
                        func=mybir.ActivationFunctionType.Identity,
                        accum_out=sums[:, bass.ds(iv, 1)])
                    junk = iop.tile([C, 512], BF16, tag="junk", name="junk")
                    nc.scalar.activation(
                        out=junk[:], in_=mm[:],
                        func=mybir.ActivationFunctionType.Square,
                        accum_out=ssums[:, bass.ds(iv, 1)])

            def stats_allreduce(gbt, st_in, st_out):
                """Returns (scale, nbias) [C,1] tiles from sums/ssums + AllReduce."""
                loc = pp.tile([C, 2], F32, name="loc", uniquify=True)
                nc.vector.reduce_sum(loc[:, 0:1], sums[:], axis=mybir.AxisListType.X)
                nc.vector.reduce_sum(loc[:, 1:2], ssums[:], axis=mybir.AxisListType.X)
                nc.gpsimd.dma_start(out=st_in[:], in_=loc[:])
                nc.gpsimd.collective_compute(
                    "AllReduce", mybir.AluOpType.add,
                    replica_groups=[list(range(NCORES))],
                    ins=[st_in[:]], outs=[st_out[:]])
                glob = pp.tile([C, 2], F32, name="glob", uniquify=True)
                nc.gpsimd.dma_start(out=glob[:], in_=st_out[:])
                # mu = sum/N ; var = ss/N - mu^2 ; scale = gamma/sqrt(var+eps)
                # nbias = beta - mu*scale
                sc = pp.tile([C, 6], F32, name="sc", uniquify=True)
                nc.scalar.activation(out=sc[:, 0:1], in_=glob[:, 0:1],
                                     func=mybir.ActivationFunctionType.Identity,
                                     scale=1.0 / N)  # mu
                nc.scalar.activation(out=sc[:, 1:2], in_=glob[:, 1:2],
                                     func=mybir.ActivationFunctionType.Identity,
                                     scale=1.0 / N)  # E[x^2]
                nc.scalar.activation(out=sc[:, 2:3], in_=sc[:, 0:1],
                                     func=mybir.ActivationFunctionType.Square)
                nc.vector.tensor_tensor(out=sc[:, 3:4], in0=sc[:, 1:2],
                                        in1=sc[:, 2:3],
                                        op=mybir.AluOpType.subtract)  # var
                nc.vector.tensor_scalar_add(sc[:, 3:4], sc[:, 3:4], EPS)
                nc.scalar.activation(out=sc[:, 4:5], in_=sc[:, 3:4],
                                     func=mybir.ActivationFunctionType.Sqrt)
                nc.vector.reciprocal(sc[:, 5:6], sc[:, 4:5])  # rstd
                res = pp.tile([C, 2], F32, name="res", uniquify=True)
                nc.vector.tensor_tensor(out=res[:, 0:1], in0=gbt[:, 0:1],
                                        in1=sc[:, 5:6],
                                        op=mybir.AluOpType.mult)  # scale
                tmp = pp.tile([C, 1], F32, name="tmp", uniquify=True)
                nc.vector.tensor_tensor(out=tmp[:], in0=sc[:, 0:1],
                                        in1=res[:, 0:1], op=mybir.AluOpType.mult)
                nc.vector.tensor_tensor(out=res[:, 1:2], in0=gbt[:, 1:2],
                                        in1=tmp[:], op=mybir.AluOpType.subtract)
                return res

            # ---------------- conv1 ----------------
            conv_pass(xpad[:], xself[:], idx1, w1t)
            res1 = stats_allreduce(gb1t, st_in1, st_out1)

            # normalize + leaky + transpose to rows -> h1shard
            for t in range(GROUPS):
                z = iop.tile([C, 512], F32, tag="z", name=f"z{t}")
                nc.scalar.activation(out=z[:], in_=raw[:, t * 512:(t + 1) * 512],
                                     func=mybir.ActivationFunctionType.Identity,
                                     scale=res1[:, 0:1], bias=res1[:, 1:2])
                zs = iop.tile([C, 512], F32, tag="zs", name=f"zs{t}")
                nc.vector.tensor_scalar_mul(zs[:], z[:], SLOPE)
                h1n = iop.tile([C, 512], BF16, tag="h1n", name=f"h1n{t}")
                nc.vector.tensor_tensor(out=h1n[:], in0=z[:], in1=zs[:],
                                        op=mybir.AluOpType.max)
                h1r = iop.tile([128, JT * C], BF16, tag="h1r", name=f"h1r{t}")
                for j in range(JT):
                    wp = wpp.tile([128, C], BF16, tag="wp", name=f"wp{t}_{j}")
                    nc.tensor.matmul(out=wp[:], lhsT=h1n[:, j * 128:(j + 1) * 128],
                                     rhs=ident[:C, :C], is_transpose=True,
                                     start=True, stop=True)
                    if j % 2 == 0:
                        nc.vector.tensor_copy(out=h1r[:, j * C:(j + 1) * C], in_=wp[:])
                    else:
                        nc.scalar.activation(
                            out=h1r[:, j * C:(j + 1) * C], in_=wp[:],
                            func=mybir.ActivationFunctionType.Identity)
                nc.sync.dma_start(
                    out=h1shard[t * 512:(t + 1) * 512, :].rearrange(
                        "(j p) c -> p j c", p=128),
                    in_=h1r[:].rearrange("p (j c) -> p j c", c=C))

            # AllGather h1 then copy into padded gather table with zero row
            nc.gpsimd.collective_compute(
                "AllGather", mybir.AluOpType.bypass,
                replica_groups=[list(range(NCORES))],
                ins=[h1shard[:]], outs=[h1full[:]])
            zrow = pp.tile([1, C], BF16)
            nc.vector.memset(zrow[:], 0.0)
            nc.sync.dma_start(out=h1tab[N:N + 1, :], in_=zrow[:])
            nc.sync.dma_start(out=h1tab[0:N, :], in_=h1full[:])

            # ---------------- conv2 ----------------
            conv_pass(h1tab[:], h1shard[:], idx2, w2t)
            res2 = stats_allreduce(gb2t, st_in2, st_out2)

            # norm2 + residual + leaky -> out
            for t in range(GROUPS):
                z = iop.tile([C, 512], F32, tag="z2", name=f"z2{t}")
                nc.scalar.activation(out=z[:], in_=raw[:, t * 512:(t + 1) * 512],
                                     func=mybir.ActivationFunctionType.Identity,
                                     scale=res2[:, 0:1], bias=res2[:, 1:2])
                xr = iop.tile([C, 512], F32, tag="xr", name=f"xr{t}")
                nc.sync.dma_start(out=xr[:], in_=xt[:, t * 512:(t + 1) * 512])
                r = iop.tile([C, 512], F32, tag="r", name=f"r{t}")
                nc.vector.tensor_add(r[:], z[:], xr[:])
                rs = iop.tile([C, 512], F32, tag="rs", name=f"rs{t}")
                nc.vector.tensor_scalar_mul(rs[:], r[:], SLOPE)
                o = iop.tile([C, 512], F32, tag="o", name=f"o{t}")
                nc.vector.tensor_tensor(out=o[:], in0=r[:], in1=rs[:],
                                        op=mybir.AluOpType.max)
                nc.sync.dma_start(out=out[:, t * 512:(t + 1) * 512], in_=o[:])

    nc.compile()
    return nc


def prep_inputs(x, W1, W2, gamma1, beta1, gamma2, beta2, nbr1, nbr2):
    """Host-side sharding/layout. Returns list of per-core input dicts."""
    xpad = np.zeros((N + 1, C), dtype=BF)
    xpad[:N] = x.astype(BF)
    w1 = np.ascontiguousarray(
        W1.astype(BF).transpose(1, 0, 2).reshape(C, K * C))
    w2 = np.ascontiguousarray(
        W2.astype(BF).transpose(1, 0, 2).reshape(C, K * C))
    gb1 = np.stack([gamma1, beta1], axis=1).astype(np.float32)
    gb2 = np.stack([gamma2, beta2], axis=1).astype(np.float32)

    def prep_idx(nbr):
        cl = np.minimum(np.asarray(nbr), ZROW).astype(np.int32)   # [K, N]

        # -> [cores, groups*128, K*JT] with slot s = k*JT + j mapping
        #    point = c*SHARD + t*512 + j*128 + p
        a = cl.reshape(K, NCORES, GROUPS, JT, 128)
        return np.ascontiguousarray(a.transpose(1, 2, 4, 0, 3)).reshape(
            NCORES, GROUPS * 128, SLOT)

    i1 = prep_idx(nbr1)
    i2 = prep_idx(nbr2)
    xtc = np.ascontiguousarray(
        x.reshape(NCORES, SHARD, C).transpose(0, 2, 1)).astype(np.float32)

    xbf = xpad[:N].reshape(NCORES, SHARD, C)
    maps = []
    for c in range(NCORES):
        maps.append({
            "xpad": xpad, "xself": np.ascontiguousarray(xbf[c]),
            "w1": w1, "w2": w2,
            "gb1": gb1, "gb2": gb2,
            "idx1": i1[c], "idx2": i2[c], "xt": xtc[c],
        })
    return maps


_CACHE = {}


def _get_runner():
    if "r" not in _CACHE:
        from runner_embedded import SpmdRunner
        nc = build_program()
        _CACHE["r"] = SpmdRunner(nc, NCORES)
    return _CACHE["r"]


def kernel(x, W1, W2, gamma1, beta1, gamma2, beta2, nbr1, nbr2):
    x = np.asarray(x, dtype=np.float32)
    maps = prep_inputs(np.asarray(x, np.float32), np.asarray(W1, np.float32),
                       np.asarray(W2, np.float32),
                       np.asarray(gamma1, np.float32), np.asarray(beta1, np.float32),
                       np.asarray(gamma2, np.float32), np.asarray(beta2, np.float32),
                       np.asarray(nbr1), np.asarray(nbr2))
    r = _get_runner()
    res = r.run(r.stage(maps))
    outs = [res[c]["out"] for c in range(NCORES)]          # [C, SHARD] each
    full = np.concatenate([o.T for o in outs], axis=0)     # [N, C]
    return np.ascontiguousarray(full.astype(np.float32))


# ---- embedded minimal SPMD runner (kernel.py must be self-contained) ----
import types

runner_embedded = types.ModuleType("runner_embedded")
exec(
    '''
import time
import numpy as np
import jax
from jax.sharding import Mesh, PartitionSpec, NamedSharding
from jax.experimental.shard_map import shard_map
import concourse.bass as bass
import concourse.mybir as mybir
from concourse import bass2jax
from concourse.bass2jax import _bass_exec_p


class SpmdRunner:
    def __init__(self, nc, n_cores):
        bass2jax.install_neuronx_cc_hook()
        self.nc = nc
        self.n_cores = n_cores
        partition_name = nc.partition_id_tensor.name if nc.partition_id_tensor else None
        in_names, out_names, out_avals = [], [], []
        for alloc in nc.m.functions[0].allocations:
            if not isinstance(alloc, mybir.MemoryLocationSet):
                continue
            name = alloc.memorylocations[0].name if alloc.memorylocations else None
            if alloc.kind == "ExternalInput":
                if name != partition_name:
                    in_names.append(name)
            elif alloc.kind == "ExternalOutput":
                out_names.append(name)
                out_avals.append(jax.core.ShapedArray(
                    tuple(alloc.tensor_shape), mybir.dt.np(alloc.dtype)))
        self.in_names, self.out_names, self.out_avals = in_names, out_names, out_avals

        def _body(*args):
            operands = list(args)
            if partition_name is not None:
                operands.append(bass2jax.partition_id_tensor())
            bind_names = list(in_names) + ([partition_name] if partition_name else [])
            outs = _bass_exec_p.bind(
                *operands, out_avals=tuple(out_avals), in_names=tuple(bind_names),
                out_names=tuple(out_names), lowering_input_output_aliases=(),
                sim_require_finite=True, sim_require_nnan=True, nc=nc)
            return tuple(outs)

        devices = jax.devices()[:n_cores]
        assert len(devices) == n_cores
        self.mesh = Mesh(np.asarray(devices), ("core",))
        in_specs = (PartitionSpec("core"),) * len(in_names)
        out_specs = (PartitionSpec("core"),) * len(out_names)
        self.fn = jax.jit(
            shard_map(_body, mesh=self.mesh, in_specs=in_specs,
                      out_specs=out_specs, check_rep=False),
            keep_unused=True)

    def stage(self, in_maps):
        sharding = NamedSharding(self.mesh, PartitionSpec("core"))
        cat = []
        for name in self.in_names:
            arrs = [np.asarray(m[name]) for m in in_maps]
            cat.append(np.concatenate(arrs, axis=0))
        return [jax.device_put(a, sharding) for a in cat]

    def run(self, staged):
        outs = self.fn(*staged)
        jax.block_until_ready(outs)
        return [
            {name: np.asarray(outs[i]).reshape(
                self.n_cores, *self.out_avals[i].shape)[c]
             for i, name in enumerate(self.out_names)}
            for c in range(self.n_cores)]

    def time(self, staged, iters=10, warmup=2):
        for _ in range(warmup):
            jax.block_until_ready(self.fn(*staged))
        t0 = time.perf_counter()
        outs = None
        for _ in range(iters):
            outs = self.fn(*staged)
        jax.block_until_ready(outs)
        t1 = time.perf_counter()
        return (t1 - t0) / iters
''',
    runner_embedded.__dict__,
)
sys.modules["runner_embedded"] = runner_embedded

